# revision 22
# baseline (speedup 1.0000x reference)
"""Trainium2 Bass kernel for nn_BasicClassifier (spiking conv classifier).

Sharding: pure data parallelism — batch 256 is split 32 samples per core
across 8 NeuronCores; params are replicated (tiny).

Per-core design. The T=1000 LIF scan is sequential, so per-step cost on the
pacing engine decides everything. State lives in a ring of [128, 128] fp32
SBUF slices (2 block-sized ring tiles, 16 slices each):
  cols 0:96   layer-1 membrane, feature f = g*128+p at (p, g*32+b), g in 0..2
  cols 96:128 layer-2 membrane [35 units x 32 samples], lagged SKEW=32 ticks
Per tick the whole LIF update (leak + input + reset-by-subtraction) is ONE
fused custom-DVE op:  m' = (m*0.9 + c) - (m > 1)   (the spike derives from
state, so the DVE chain never waits on other engines mid-block). Its input
drive c is read straight out of a 4-bank PSUM block tile via a strided
[128, 4, 32] access pattern — no assembly copies.

Per 16-tick block (all off the tick-critical path, bf16 hi/lo splits keep
matmuls single-pass while preserving ~2^-16 relative precision):
  - C tile = PSUM [128, 4*512]: banks 0-2 = conv1d hoisted into [31 -> 384]
    GEMMs (3 products: xh@Wh + xl@Wh + xh@Wl; ones row folds conv_b),
    bank 3 = fc bias prefill (bh+bl) + fc results of block b-2.
  - spikes: ONE ACT Sign op per block over the ring -> sigma in {-1,0,1} bf16;
    (sigma+1)/2 is folded into halved fc weights + adjusted fc bias.
  - fc 384->35: 6 bf16 matmuls of N=512 (hi/lo x 3 K-chunks) accumulating
    onto the bias in bank 3 of the C tile two blocks ahead.
  - mem2 history: ACT copies ring cols 96:128 to an fp16 [35, 32*T] buffer;
    one final DVE tensor_reduce produces sum_t mem2.
"""

import os
import sys

for _p in ("/opt/trn_rl_repo", "/opt/pypackages"):
    if _p not in sys.path:
        sys.path.insert(0, _p)

import numpy as np

import concourse.bacc as bacc
import concourse.mybir as mybir
import concourse.tile as tile
import concourse.dve_ops as dve_ops
from concourse.dve_spec import Spec, Src0, Src1, C0, C1, lower
from concourse.dve_uop import DveOpSpec
from concourse.bass_utils import run_bass_kernel_spmd

F32 = mybir.dt.float32
F16 = mybir.dt.float16
BF16 = mybir.dt.bfloat16
ALU = mybir.AluOpType
AF = mybir.ActivationFunctionType
AX = mybir.AxisListType

N_CORES = 8
B_FULL, T_FULL, L_IN = 256, 1000, 30
BC = B_FULL // N_CORES      # 32 samples per core
CH, LO = 16, 24
F = CH * LO                 # 384 features
G = 3                       # feature groups of 128
J = 35                      # fc outputs
KX = L_IN + 1               # conv contraction rows (30 taps + ones row)
BLK = 16                    # ticks per block (N = 16*32 = 512 = 1 PSUM bank)
SKEW = 3 * BLK              # layer-2 lag: c2_t consumed at DVE tick t+SKEW
WIN = 160                   # ticks per x-window DMA (multiple of BLK)
BETA, THR = 0.9, 1.0

TRACE = bool(int(os.environ.get("KERNEL_TRACE", "0")))
LAST_RESULTS = None

_LIF_OP = None


def _get_lif_op():
    """Register the fused LIF-step op in the custom-DVE table (idempotent)."""
    global _LIF_OP
    if _LIF_OP is not None:
        return _LIF_OP
    name = "LIF_STEP_ANT59"
    for op in dve_ops.OPS:
        if op.name == name:
            _LIF_OP = op
            return op
    spec = Spec(
        body=(Src0 * C0 + Src1) - (Src0 > C1),
        reference=lambda in0, in1, s0, s1, imm2: (
            (in0.astype(np.float32) * np.float32(s0)
             + in1.reshape(in0.shape))
            - (in0 > s1).astype(np.float32)
        ).astype(np.float32),
    )
    row = dve_ops._CUSTOM_DVE_ROW_BASE + len(dve_ops.OPS)
    assert row < 0x20
    dve_ops._SUB_OPCODE_FOR_NAME[name] = row
    shas = {}
    for ver in ("v3", "v4"):
        tmp = DveOpSpec(name=name, opcode=row, uops=lower(spec, ver=ver), rd1_en=True)
        shas[ver] = tmp.sha(ver)
    op = dve_ops.DveOp(name, spec, subdim=False, uops_sha=shas)
    dve_ops.OPS.append(op)
    dve_ops.CUSTOM_DVE_SPECS[name] = spec
    _LIF_OP = op
    return op


def _build_nc(T):
    """Build the per-core Bass program (SPMD: same program on every core)."""
    lif = _get_lif_op()
    ticks = T + SKEW                       # DVE ticks 0..T+SKEW-1
    nblk = -(-ticks // BLK)
    pad_ticks = nblk * BLK
    windows = -(-pad_ticks // WIN)
    xt_cols = windows * WIN * BC
    fcblk = -(-T // BLK)                   # blocks that need spikes/fc
    NB = BLK * BC                          # 512: one PSUM bank of f32

    nc = bacc.Bacc("TRN2", target_bir_lowering=False)

    KS = 3 * KX                            # stacked conv K: [xh; xl; xh]
    xts_d = nc.dram_tensor("xts", [KS, xt_cols], BF16, kind="ExternalInput")
    wes_d = nc.dram_tensor("wes", [KS, F], BF16, kind="ExternalInput")
    fch_d = nc.dram_tensor("fch", [128, G * J], BF16, kind="ExternalInput")
    fcl_d = nc.dram_tensor("fcl", [128, G * J], BF16, kind="ExternalInput")
    brs_d = nc.dram_tensor("brs", [2, 128], BF16, kind="ExternalInput")
    ones_d = nc.dram_tensor("ones", [2, NB], BF16, kind="ExternalInput")
    hist_d = nc.dram_tensor("hist", [J, BC * T], F32, kind="ExternalOutput")

    with tile.TileContext(nc) as tc:
        with (
            tc.tile_pool(name="konst", bufs=1) as kp,
            tc.tile_pool(name="ring", bufs=1) as rp,
            tc.tile_pool(name="csb", bufs=2) as sp2,
            tc.tile_pool(name="sig", bufs=2) as sgp,
            tc.tile_pool(name="xwin", bufs=2) as xp,
            tc.tile_pool(name="cpsum", bufs=2, space="PSUM") as cp,
        ):
            # constants -> SBUF
            wes = kp.tile([KS, F], BF16, tag="wes")
            fch = kp.tile([128, G * J], BF16, tag="fch")
            fcl = kp.tile([128, G * J], BF16, tag="fcl")
            brs = kp.tile([2, 128], BF16, tag="brs")
            ones = kp.tile([2, NB], BF16, tag="ones")
            negthr = kp.tile([128, 1], F32, tag="negthr")
            nc.vector.memset(negthr[:], -THR)
            for sb, dr in ((wes, wes_d), (fch, fch_d), (fcl, fcl_d),
                           (brs, brs_d), (ones, ones_d)):
                nc.sync.dma_start(sb[:], dr[:])

            # state ring: 2 block-sized tiles of 16 slices each
            ringA = rp.tile([128, BLK * 128], F32, tag="ringA")
            ringB = rp.tile([128, BLK * 128], F32, tag="ringB")
            nc.vector.memset(ringA[:], 0.0)
            nc.vector.memset(ringB[:], 0.0)
            rings = (ringA, ringB)


            xts = {}      # window idx -> xt sbuf tile
            chs = {}      # block idx -> PSUM C tile [128, 4*512]
            sbs = {}      # block idx -> SBUF C tile [128, BLK*128], tick-major
            convd = set()

            def load_window(w):
                if w >= windows or w in xts:
                    return
                ts = xp.tile([KS, WIN * BC], BF16, tag="xws")
                nc.sync.dma_start(ts[:], xts_d[:, w * WIN * BC:(w + 1) * WIN * BC])
                xts[w] = ts

            def ensure_psum(b):
                """Allocate block b's PSUM C tile and prime bank 3 with the
                fc bias (or zeros for blocks whose c2 source predates t=0)."""
                if b >= nblk or b in chs:
                    return
                ch = cp.tile([128, 4 * NB], F32, tag="ch")
                chs[b] = ch
                if b >= 3:
                    nc.tensor.matmul(
                        out=ch[:, G * NB:4 * NB],
                        lhsT=brs[:, :], rhs=ones[:, :],
                        start=True, stop=False,
                        skip_group_check=True,
                    )
                else:
                    nc.vector.memset(ch[:, G * NB:4 * NB], 0.0)

            def conv_mm(b, g):
                """One conv GEMM into bank g of block b's PSUM tile."""
                if b >= nblk or (b, g) in convd:
                    return
                convd.add((b, g))
                w = (b * BLK) // WIN
                base = (b * BLK - w * WIN) * BC
                nc.tensor.matmul(
                    out=chs[b][:, g * NB:(g + 1) * NB],
                    lhsT=wes[:, g * 128:(g + 1) * 128],
                    rhs=xts[w][:, base:base + NB],
                    start=True, stop=True,
                )

            def sigma(b):
                """sigma = Sign(m1 - 1) over block b's ring tile (3 ACT ops,
                contiguous per-g outputs). Returns the sigma tile."""
                ring3 = rings[b % 2][:].rearrange("p (t c) -> p t c", c=128)
                sg = sgp.tile([128, G * NB], BF16, tag="sg")
                for g in range(G):
                    nc.scalar.activation(
                        out=sg[:, g * NB:(g + 1) * NB],
                        in_=ring3[:, :, g * BC:(g + 1) * BC], func=AF.Sign,
                        bias=negthr[:],
                    )
                return sg

            def fc_and_conv(b):
                """fc of block b-1's spikes (into PSUM tile b+2 bank 3),
                interleaved with conv fills of tile b+1 to hide the PSUM
                accumulate interlock between same-bank fc matmuls."""
                fb = b - 1
                do_fc = 0 <= fb < fcblk
                sg = sigma(fb) if do_fc else None
                fcmm = []
                if do_fc:
                    i = 0
                    for g in range(G):
                        for lw in (fch, fcl):
                            i += 1
                            fcmm.append((lw[:, g * J:(g + 1) * J],
                                         sg[:, g * NB:(g + 1) * NB], i == 2 * G))
                k = 0
                for lhsT, rhs, stop in fcmm:
                    nc.tensor.matmul(
                        out=chs[fb + 3][0:J, G * NB:4 * NB],
                        lhsT=lhsT, rhs=rhs, start=False, stop=stop,
                        skip_group_check=True,
                    )
                    if k < G:
                        conv_mm(b + 1, k)
                        k += 1
                while k < G:
                    conv_mm(b + 1, k)
                    k += 1

            def sbuf_copy(b):
                """ACT: copy block b's complete PSUM C tile into the
                tick-major SBUF layout the DVE op consumes."""
                if b >= nblk or b in sbs:
                    return
                cs = sp2.tile([128, BLK * 128], F32, tag="cs")
                sbs[b] = cs
                src = chs[b][:].rearrange("p (g t n) -> p g t n", g=4, n=BC)
                dst = cs[:].rearrange("p (t g n) -> p g t n", g=4, n=BC)
                nc.scalar.activation(out=dst, in_=src, func=AF.Copy)

            def hist_dma(b):
                """mem2 of DVE-tick block b = m2 ticks [16b-SKEW, ...):
                DMA straight from the ring to DRAM (host sums)."""
                t0 = b * BLK - SKEW
                if t0 < 0:
                    return
                n = min(BLK, T - t0)
                if n <= 0:
                    return
                ring3 = rings[b % 2][:].rearrange("p (t c) -> p t c", c=128)
                nc.sync.dma_start(
                    hist_d[:, t0 * BC:(t0 + n) * BC],
                    ring3[0:J, 0:n, G * BC:128],
                )

            # prologue
            load_window(0)
            ensure_psum(0)
            ensure_psum(1)
            for g in range(G):
                conv_mm(0, g)
            sbuf_copy(0)

            for t in range(ticks):
                b, lo = divmod(t, BLK)
                if lo == 0:
                    if ((b + 3) * BLK) % WIN == 0:
                        load_window(((b + 3) * BLK) // WIN)
                    ensure_psum(b + 2)
                    fc_and_conv(b)
                    sbuf_copy(b + 1)
                    hist_dma(b - 1)
                ring = rings[b % 2]
                prev = rings[(b - 1) % 2] if lo == 0 else ring
                plo = (lo - 1) % BLK

                nc.vector._custom_dve(
                    lif,
                    out=ring[:, lo * 128:(lo + 1) * 128],
                    in0=prev[:, plo * 128:(plo + 1) * 128],
                    in1=sbs[b][:, lo * 128:(lo + 1) * 128],
                    s0=BETA, s1=THR,
                )

            # epilogue: the last block's mem2 history
            hist_dma(nblk - 1)



    nc.compile()
    return nc


def _bf16_split(a):
    import ml_dtypes
    hi = a.astype(ml_dtypes.bfloat16)
    lo = (a - hi.astype(np.float32)).astype(ml_dtypes.bfloat16)
    return hi, lo


def _host_prep(x, conv_w, conv_b, fc_w, fc_b, T):
    """Build per-core input maps (numpy only)."""
    import ml_dtypes
    ticks = T + SKEW
    nblk = -(-ticks // BLK)
    windows = -(-(nblk * BLK) // WIN)
    xt_ticks = windows * WIN

    wexp = np.zeros((KX, F), np.float32)
    for c in range(CH):
        for l in range(LO):
            wexp[l:l + 7, c * LO + l] = conv_w[c, 0, :]
        wexp[L_IN, c * LO:(c + 1) * LO] = conv_b[c]
    weh, wel = _bf16_split(wexp)
    wes = np.concatenate([weh, weh, wel], axis=0)  # K-stacked [93, F]

    # spike trick: s = (sigma+1)/2 with sigma = sign(m-1) in {-1,0,1}
    # c2 = fc_w @ s + b = (fc_w/2) @ sigma + (b + fc_w.sum/2)
    half = (fc_w * 0.5).astype(np.float32)
    fcwt = np.zeros((128, G * J), np.float32)
    for g in range(G):
        fcwt[:, g * J:(g + 1) * J] = half[:, g * 128:(g + 1) * 128].T
    fch, fcl = _bf16_split(fcwt)
    brow = np.zeros((1, 128), np.float32)
    brow[0, :J] = fc_b + half.sum(axis=1)
    brh, brl = _bf16_split(brow)
    brs = np.concatenate([brh, brl], axis=0)       # [2, 128]

    ones = np.ones((2, BLK * BC), ml_dtypes.bfloat16)

    in_maps = []
    B = x.shape[0]
    n_cores = B // BC
    for core in range(n_cores):
        xc = x[core * BC:(core + 1) * BC]          # [BC, T, L]
        xt = np.zeros((KX, xt_ticks, BC), np.float32)
        xt[:L_IN, :T, :] = xc.transpose(2, 1, 0)
        xt[L_IN, :T, :] = 1.0
        xt = xt.reshape(KX, xt_ticks * BC)
        xth, xtl = _bf16_split(xt)
        xstk = np.concatenate([xth, xtl, xth], axis=0)  # [93, cols]
        in_maps.append({
            "xts": xstk, "wes": wes, "fch": fch, "fcl": fcl,
            "brs": brs, "ones": ones,
        })
    return in_maps


def _install_trace_hook():
    """Wire up the axon NTFF profiling hook (absent from this image)."""
    import types

    if "antenv.axon_hooks" in sys.modules:
        return True
    try:
        if "/root/.axon_site" not in sys.path:
            sys.path.insert(0, "/root/.axon_site")
        from trn_agent_boot.trn_boot import _ntff_profile_via_ctypes

        hook = _ntff_profile_via_ctypes("/opt/axon/libaxon_pjrt.so")
        if hook is None:
            return False
        mod = types.ModuleType("antenv.axon_hooks")
        mod.get_axon_ntff_profile_hook = lambda: hook
        sys.modules["antenv.axon_hooks"] = mod
        import concourse.bass_utils as bu

        bu.upload_artifacts = lambda tmpdir: str(tmpdir)
        return True
    except Exception as e:  # profiling is optional
        print(f"trace hook install failed: {e}", file=sys.stderr)
        return False


def run_cores(x, conv_w, conv_b, fc_w, fc_b, T=None):
    """Run the Bass kernel on len(batch)/32 cores; returns [B, 35] output."""
    global LAST_RESULTS
    T = T if T is not None else x.shape[1]
    trace = TRACE and _install_trace_hook()
    nc = _build_nc(T)
    in_maps = _host_prep(x, conv_w, conv_b, fc_w, fc_b, T)
    res = run_bass_kernel_spmd(
        nc, in_maps, core_ids=list(range(len(in_maps))), trace=trace,
    )
    LAST_RESULTS = res
    outs = []
    for i in range(len(in_maps)):
        hv = np.asarray(res.results[i]["hist"], dtype=np.float32)
        m2 = hv.reshape(J, T, BC)                  # [J, t, sample]
        outs.append((m2.sum(axis=1) / np.float32(T)).T.astype(np.float32))
    return np.concatenate(outs, axis=0)


def kernel(x, conv_w, conv_b, fc_w, fc_b):
    return run_cores(
        np.asarray(x, np.float32), np.asarray(conv_w, np.float32),
        np.asarray(conv_b, np.float32), np.asarray(fc_w, np.float32),
        np.asarray(fc_b, np.float32),
    )
